# revision 25
# baseline (speedup 1.0000x reference)
"""DepthGatedModule kernel for 8 Trainium2 NeuronCores (Bass/Tile).

Reference computation (B=4, C=512, H=W=48, N=B*H*W=9216 tokens):
  xt  = tok(x) @ w_rgb.T + b_rgb
  lhs = tok(d) @ w_lhs.T + b_lhs ; rhs = tok(d) @ w_rhs.T + b_rhs
  P   = softmax(lhs @ rhs.T, axis=1) ;  enh = P @ xt
  y   = enh @ w_dec.T + b_dec ;  BatchNorm(train) ; ReLU

Sharding: queries split 1152/core, keys/values replicated.

The per-key linears are eliminated algebraically:
  - S = lhs @ rhs.T = (lhs @ w_rhs) @ d.T: fold w_rhs into the query side
    (lhs2 = lhs @ w_rhs, a Q-local linear) and use RAW d as keys. b_rhs
    adds a per-query constant to logits, which softmax cancels - dropped.
  - P @ (x @ w_rgb.T) = (P @ x) @ w_rgb.T: PV runs on RAW x; w_rgb then
    folds into the decoder on the host (w_comb = w_dec @ w_rgb,
    b_dec2 = w_dec @ b_rgb + b_dec, exact since softmax rows sum to 1).
  This removes all replicated per-key matmuls (~40% of tensor work).

Precision (gate 2e-2; softmax amplifies logit noise ~1:1 into output
relative error, so the logits chain needs ~10+ mantissa bits; the PV
chain tolerates bf16):
  - fp16 logits chain: d (S stationary - 2-byte weight loads at half the
    fp32r load time, no gpsimd cast-DMA needed), d_q, w_lhs, w_rhs,
    lhs/lhs2. fp16's 10-bit mantissa matches fp32r's precision class here
    and all values are ~N(0,1)-scaled, far from fp16 range limits.
  - bf16 PV chain: exp tiles (written directly by ACT; bf16 needed for
    exp range e^12 > fp16 max), raw x (SBUF-resident), enh, w_comb.
  Measured: rel err ~4e-3 (vs 3.3e-3 for an fp32r logits chain).

Schedule per core: query-tile outer (3 x 384), key loop inner; PV
accumulates in PSUM across all 72 key blocks (start/stop spanning the
whole loop); S and PV software-pipelined with a 2-tile skew so the
tensor engine never waits on ACT exp. Softmax denominator = vector/
gpsimd adds of exp tiles + one tiny matmul per 128-query block. Each
query tile's epilogue (normalize from PSUM, transpose, decoder, BN
partial sums) is spread in closure-steps over the next tile's first
chunk iterations. BatchNorm stats go through two 4KB AllReduces: qt0+qt1
during qt2's attention (qt2 streams d via the sync queue since the
collective blocks gpsimd), qt2's on the tail.
"""
import numpy as np
import ml_dtypes

import concourse.bacc as bacc
import concourse.bass as bass
import concourse.mybir as mybir
import concourse.tile as tile
from concourse.bass_utils import run_bass_kernel_spmd
from concourse.masks import make_identity

F32 = mybir.dt.float32
FP16 = mybir.dt.float16
BF16 = mybir.dt.bfloat16
AF = mybir.ActivationFunctionType

B, C, H, W = 4, 512, 48, 48
N = B * H * W            # 9216 tokens
NCORES = 8
Q = N // NCORES          # 1152 queries per core
CB = C // 128            # 4 channel blocks
KC = 512                 # key-chunk tokens
NKC = N // KC            # 18 key chunks
KB = KC // 128           # 4 key blocks per chunk
QT = 384                 # query tile (free dim of S^T matmuls)
NQT = Q // QT            # 3 query tiles
QB = QT // 128           # 3 query blocks per tile
SHIFT = -40.0            # softmax constant shift (logit max ~52 + |lhs.b_rhs|)
BN_EPS = 1e-5

_nc_cache = None


def _build(use_collective=True):
    nc = bacc.Bacc("TRN2", target_bir_lowering=False, debug=False,
                   num_devices=NCORES)

    d_q = nc.declare_dram_parameter("d_q", [C, Q], FP16, isOutput=False)
    d_full = nc.declare_dram_parameter("d_full", [C, N], FP16, isOutput=False)
    x_tm = nc.declare_dram_parameter("x_tm", [N, C], BF16, isOutput=False)
    wT_lhs = nc.declare_dram_parameter("wT_lhs", [C, C], FP16, isOutput=False)
    w_rhs_om = nc.declare_dram_parameter("w_rhs_om", [C, C], FP16,
                                         isOutput=False)
    wT_comb = nc.declare_dram_parameter("wT_comb", [C, C], BF16, isOutput=False)
    vecs = {
        name: nc.declare_dram_parameter(name, [C], F32, isOutput=False)
        for name in ["b_lhs", "b_dec2", "gamma", "beta"]
    }
    y_out = nc.declare_dram_parameter("y", [C, Q], F32, isOutput=True)

    dq_re = d_q.rearrange("(cb p) n -> p cb n", p=128)
    d_re = d_full.rearrange("(cb p) n -> p cb n", p=128)
    x_re = x_tm.rearrange("(tb p) c -> p tb c", p=128)
    y_re = y_out.rearrange("(cb p) n -> p cb n", p=128)

    with tile.TileContext(nc) as tc:
        with (
            tc.tile_pool(name="consts", bufs=1) as consts,
            tc.tile_pool(name="res", bufs=1) as res,
            tc.tile_pool(name="mm", bufs=2, space="PSUM") as mmp,
            tc.tile_pool(name="enh", bufs=2, space="PSUM") as enhp,
            tc.tile_pool(name="dram", bufs=1, space="DRAM") as dram,
        ):
            # ---- constants / weights (parallel DMA queues) ----
            wl_t = consts.tile([128, CB, C], FP16, tag="wl")
            nc.gpsimd.dma_start(out=wl_t[:, :, :],
                                in_=wT_lhs.rearrange("(cb p) co -> p cb co", p=128))
            wr_t = consts.tile([128, CB, C], FP16, tag="wr")
            nc.gpsimd.dma_start(out=wr_t[:, :, :],
                                in_=w_rhs_om.rearrange("(ob p) i -> p ob i", p=128))
            wc_t = consts.tile([128, CB, C], BF16, tag="wc")
            nc.scalar.dma_start(out=wc_t[:, :, :],
                                in_=wT_comb.rearrange("(jb p) o -> p jb o", p=128))
            v_t = {}
            for name in vecs:
                v = consts.tile([128, CB], F32, tag=f"v_{name}")
                nc.scalar.dma_start(out=v[:, :],
                                    in_=vecs[name].rearrange("(cb p) -> p cb", p=128))
                v_t[name] = v

            shift_t = consts.tile([128, 1], F32)
            nc.vector.memset(shift_t, SHIFT)
            ones_f = consts.tile([128, 1], F32)
            nc.vector.memset(ones_f, 1.0)
            ident_f = consts.tile([128, 128], F32)
            make_identity(nc, ident_f[:, :])
            ident_bf = consts.tile([128, 128], BF16)
            nc.vector.tensor_copy(out=ident_bf[:, :], in_=ident_f[:, :])

            # ---- persistent SBUF state ----
            lhs2 = res.tile([128, CB, Q], FP16)      # (lhs @ w_rhs) ch-major
            # sum of exp tiles; plane 0 fed by vector, plane 1 by gpsimd
            esum = res.tile([128, NQT, 2, QT], F32)
            nc.vector.memset(esum[:, :, :, :], 0.0)
            esum_t = res.tile([128, NQT, QT], F32)   # combined
            den_sb = res.tile([128, NQT * 4], F32)
            rden = res.tile([128, NQT * 4], F32)
            enh_cm = res.tile([128, CB, Q], BF16)    # normalized, channel-major
            y_sb = res.tile([128, CB, Q], F32)       # decoder out, pre-BN
            sums3 = res.tile([128, NQT, 2 * CB], F32)  # per-qt BN partials
            sums_a = res.tile([128, 2 * CB], F32)    # qt0+qt1 combined
            gs_a = res.tile([128, 2 * CB], F32)      # early-AllReduce result
            ar_in_a = dram.tile([128, 2 * CB], F32)
            # Shared-HBM collective outputs skip a local bounce copy
            ar_out_a = nc.dram_tensor("ar_out_a", [128, 2 * CB], F32,
                                      addr_space="Shared").ap()

            # ---- lhs -> lhs2 for this core's queries (scoped scratch) ----
            with tc.tile_pool(name="tmpq", bufs=1) as tmpq:
                dq_sb = tmpq.tile([128, CB, Q], FP16)
                for qt in range(NQT):      # sliced so lhs can start early
                    q0 = qt * QT
                    nc.sync.dma_start(out=dq_sb[:, :, q0:q0 + QT],
                                      in_=dq_re[:, :, q0:q0 + QT])
                lhs_tmp = tmpq.tile([128, CB, Q], FP16)
                for qt in range(NQT):
                    q0 = qt * QT
                    for co in range(CB):
                        ps = mmp.tile([128, 512], F32, tag="mm")
                        for ci in range(CB):
                            nc.tensor.matmul(
                                ps[:, :QT],
                                lhsT=wl_t[:, ci, co * 128:(co + 1) * 128],
                                rhs=dq_sb[:, ci, q0:q0 + QT],
                                start=(ci == 0), stop=(ci == CB - 1))
                        nc.vector.tensor_scalar_add(
                            out=lhs_tmp[:, co, q0:q0 + QT], in0=ps[:, :QT],
                            scalar1=v_t["b_lhs"][:, co:co + 1])
                for qt in range(NQT):
                    q0 = qt * QT
                    for ib in range(CB):
                        ps = mmp.tile([128, 512], F32, tag="mm")
                        for ob in range(CB):
                            nc.tensor.matmul(
                                ps[:, :QT],
                                lhsT=wr_t[:, ob, ib * 128:(ib + 1) * 128],
                                rhs=lhs_tmp[:, ob, q0:q0 + QT],
                                start=(ob == 0), stop=(ob == CB - 1))
                        nc.vector.tensor_copy(out=lhs2[:, ib, q0:q0 + QT],
                                              in_=ps[:, :QT])

            with (
                tc.tile_pool(name="xres", bufs=1) as xres,
                tc.tile_pool(name="chunks", bufs=3) as chunks,
                tc.tile_pool(name="et", bufs=4) as etp,
                tc.tile_pool(name="outp", bufs=2) as outp,
            ):
                # raw x, token-major, SBUF-resident (PV moving operand).
                # Loaded per-chunk so early PV tiles don't wait on the tail.
                x_res = xres.tile([128, NKC * KB, C], BF16)
                for kc in range(NKC):
                    nc.sync.dma_start(
                        out=x_res[:, kc * KB:(kc + 1) * KB, :],
                        in_=x_re[:, kc * KB:(kc + 1) * KB, :])

                def make_epilogue(qt, enh_ps):
                    """Per-qt epilogue as closures, spread over the next qt's
                    first chunk iterations so the tensor queue never blocks
                    on the den/normalize vector/scalar latency chain."""
                    q0 = qt * QT

                    def den_step():
                        nc.vector.tensor_add(out=esum_t[:, qt, :],
                                             in0=esum[:, qt, 0, :],
                                             in1=esum[:, qt, 1, :])
                        for qb in range(QB):
                            dps = mmp.tile([128, 1], F32, tag="mm")
                            nc.tensor.matmul(
                                dps,
                                lhsT=esum_t[:, qt, qb * 128:(qb + 1) * 128],
                                rhs=ones_f[:, :])
                            nc.scalar.copy(
                                out=den_sb[:, qt * 4 + qb:qt * 4 + qb + 1],
                                in_=dps)
                        nc.vector.reciprocal(out=rden[:, qt * 4:qt * 4 + QB],
                                             in_=den_sb[:, qt * 4:qt * 4 + QB])

                    def norm_step(qb):
                        qb9 = qt * QB + qb
                        en = outp.tile([128, C], BF16, tag="en")
                        nc.scalar.activation(
                            out=en, in_=enh_ps[:, qb, :], func=AF.Identity,
                            scale=rden[:, qt * 4 + qb:qt * 4 + qb + 1])
                        for cb in range(CB):
                            tp = mmp.tile([128, 128], BF16, tag="mm")
                            nc.tensor.transpose(
                                tp, en[:, cb * 128:(cb + 1) * 128],
                                ident_bf[:, :])
                            nc.scalar.copy(
                                out=enh_cm[:, cb, qb9 * 128:(qb9 + 1) * 128],
                                in_=tp)

                    def dec_step(cos):
                        for co in cos:
                            ps = mmp.tile([128, QT], F32, tag="mm")
                            for ci in range(CB):
                                nc.tensor.matmul(
                                    ps,
                                    lhsT=wc_t[:, ci, co * 128:(co + 1) * 128],
                                    rhs=enh_cm[:, ci, q0:q0 + QT],
                                    start=(ci == 0), stop=(ci == CB - 1))
                            nc.vector.tensor_scalar_add(
                                out=y_sb[:, co, q0:q0 + QT], in0=ps,
                                scalar1=v_t["b_dec2"][:, co:co + 1])

                    def bn_step():
                        for cb in range(CB):
                            nc.vector.reduce_sum(out=sums3[:, qt, cb:cb + 1],
                                                 in_=y_sb[:, cb, q0:q0 + QT],
                                                 axis=mybir.AxisListType.X)
                            junk = outp.tile([128, QT], F32, tag="junk")
                            nc.scalar.activation(
                                out=junk, in_=y_sb[:, cb, q0:q0 + QT],
                                func=AF.Square,
                                accum_out=sums3[:, qt, CB + cb:CB + cb + 1])

                    def ar_early():
                        # qt0+qt1 stats reduce while qt2 attention runs; this
                        # occupies the gpsimd queue, so qt2 streams d-chunks
                        # via sync and keeps its esum adds on vector.
                        nc.vector.tensor_add(out=sums_a[:, :],
                                             in0=sums3[:, 0, :],
                                             in1=sums3[:, 1, :])
                        nc.gpsimd.dma_start(out=ar_in_a[:], in_=sums_a[:, :])
                        if use_collective:
                            nc.gpsimd.collective_compute(
                                "AllReduce", mybir.AluOpType.add,
                                replica_groups=[list(range(NCORES))],
                                ins=[ar_in_a.opt()], outs=[ar_out_a.opt()])
                        else:
                            nc.gpsimd.dma_start(out=ar_out_a,
                                                in_=ar_in_a[:])

                    steps = [den_step] + [lambda qb=qb: norm_step(qb)
                                          for qb in range(QB)]
                    steps += [lambda: dec_step((0, 1)), lambda: dec_step((2, 3)),
                              bn_step]
                    if qt == 1:
                        steps.append(ar_early)
                    return steps

                epi = []
                for qt in range(NQT):
                    q0 = qt * QT
                    enh_ps = enhp.tile([128, QB, 512], F32, tag="enh")
                    last_qt = qt == NQT - 1

                    def emit_pv(pend, last):
                        pe, pki = pend
                        for qb in range(QB):
                            nc.tensor.matmul(
                                enh_ps[:, qb, :],
                                lhsT=pe[:, qb * 128:(qb + 1) * 128],
                                rhs=x_res[:, pki, :],
                                start=(pki == 0), stop=last,
                                skip_group_check=True)

                    pend = []               # [(e_bf, key-block idx), ...]
                    for kc in range(NKC):
                        if epi and kc >= 1:
                            epi.pop(0)()
                        d_ch = chunks.tile([128, CB, KC], FP16, tag="dch")
                        eng = nc.sync if last_qt else nc.gpsimd
                        eng.dma_start(out=d_ch[:, :, :],
                                      in_=d_re[:, :, kc * KC:(kc + 1) * KC])
                        for kb in range(KB):
                            ki = kc * KB + kb
                            st = mmp.tile([128, QT], F32, tag="mm")
                            for ci in range(CB):
                                nc.tensor.matmul(
                                    st,
                                    lhsT=d_ch[:, ci, kb * 128:(kb + 1) * 128],
                                    rhs=lhs2[:, ci, q0:q0 + QT],
                                    start=(ci == 0), stop=(ci == CB - 1))
                            e_bf = etp.tile([128, QT], BF16, tag="et")
                            nc.scalar.activation(out=e_bf, in_=st, func=AF.Exp,
                                                 bias=shift_t[:, :], scale=1.0)
                            if len(pend) == 2:
                                emit_pv(pend.pop(0), False)
                            pend.append((e_bf, ki))
                            plane = 0 if last_qt else (ki & 1)
                            eng2 = nc.gpsimd if plane else nc.vector
                            eng2.tensor_add(out=esum[:, qt, plane, :],
                                            in0=esum[:, qt, plane, :],
                                            in1=e_bf)
                    while epi:
                        epi.pop(0)()
                    for i, pv in enumerate(pend):
                        emit_pv(pv, i == len(pend) - 1)
                    epi = make_epilogue(qt, enh_ps)
                for f in epi:
                    f()

                # ---- BN stats AllReduce (qt2 part) + apply ----
                ar_in = dram.tile([128, 2 * CB], F32)
                ar_out = nc.dram_tensor("ar_out_b", [128, 2 * CB], F32,
                                        addr_space="Shared").ap()
                nc.gpsimd.dma_start(out=gs_a[:, :], in_=ar_out_a)
                nc.gpsimd.dma_start(out=ar_in[:], in_=sums3[:, 2, :])
                if use_collective:
                    nc.gpsimd.collective_compute(
                        "AllReduce", mybir.AluOpType.add,
                        replica_groups=[list(range(NCORES))],
                        ins=[ar_in.opt()], outs=[ar_out.opt()])
                else:
                    nc.gpsimd.dma_start(out=ar_out, in_=ar_in[:])
                gs = consts.tile([128, 2 * CB], F32)
                nc.gpsimd.dma_start(out=gs[:, :], in_=ar_out)
                nc.vector.tensor_add(out=gs[:, :], in0=gs[:, :], in1=gs_a[:, :])

                # mean/var -> scale/bias (rsqrt = sqrt + recip + 1 Newton)
                mean_t = consts.tile([128, CB], F32)
                nc.vector.tensor_scalar_mul(out=mean_t[:, :], in0=gs[:, 0:CB],
                                            scalar1=1.0 / N)
                var_t = consts.tile([128, CB], F32)
                nc.vector.tensor_scalar_mul(out=var_t[:, :], in0=gs[:, CB:2 * CB],
                                            scalar1=1.0 / N)
                m2 = consts.tile([128, CB], F32)
                nc.vector.tensor_mul(out=m2[:, :], in0=mean_t[:, :],
                                     in1=mean_t[:, :])
                nc.vector.tensor_sub(out=var_t[:, :], in0=var_t[:, :],
                                     in1=m2[:, :])
                nc.vector.tensor_scalar_add(out=var_t[:, :], in0=var_t[:, :],
                                            scalar1=BN_EPS)
                sq = consts.tile([128, CB], F32)
                nc.scalar.sqrt(out=sq[:, :], in_=var_t[:, :])
                inv0 = consts.tile([128, CB], F32)
                nc.vector.reciprocal(out=inv0[:, :], in_=sq[:, :])
                t1 = consts.tile([128, CB], F32)
                nc.vector.tensor_mul(out=t1[:, :], in0=var_t[:, :], in1=inv0[:, :])
                nc.vector.tensor_mul(out=t1[:, :], in0=t1[:, :], in1=inv0[:, :])
                nc.vector.tensor_scalar(out=t1[:, :], in0=t1[:, :],
                                        scalar1=-0.5, scalar2=1.5,
                                        op0=mybir.AluOpType.mult,
                                        op1=mybir.AluOpType.add)
                inv_t = consts.tile([128, CB], F32)
                nc.vector.tensor_mul(out=inv_t[:, :], in0=inv0[:, :], in1=t1[:, :])

                scale_t = consts.tile([128, CB], F32)
                nc.vector.tensor_mul(out=scale_t[:, :], in0=inv_t[:, :],
                                     in1=v_t["gamma"][:, :])
                bias2_t = consts.tile([128, CB], F32)
                nc.vector.tensor_mul(out=bias2_t[:, :], in0=mean_t[:, :],
                                     in1=scale_t[:, :])
                nc.vector.tensor_sub(out=bias2_t[:, :], in0=v_t["beta"][:, :],
                                     in1=bias2_t[:, :])

                st_engs = [nc.sync, nc.gpsimd, nc.scalar, nc.sync]
                for cb in range(CB):
                    yo = outp.tile([128, Q], F32, tag="yo")
                    nc.scalar.activation(out=yo, in_=y_sb[:, cb, :],
                                         func=AF.Relu,
                                         scale=scale_t[:, cb:cb + 1],
                                         bias=bias2_t[:, cb:cb + 1])
                    st_engs[cb].dma_start(out=y_re[:, cb, :], in_=yo)

    nc.finalize()
    return nc


def _prepare_in_maps(x, from_depth_estimation, w_rgb, b_rgb, w_lhs, b_lhs,
                     w_rhs, b_rhs, w_dec, b_dec, gamma, beta):
    f32 = np.float32
    bf = ml_dtypes.bfloat16
    x_tm = np.ascontiguousarray(
        np.asarray(x, dtype=f32).transpose(0, 2, 3, 1).reshape(N, C)
        .astype(bf))
    d_cm = np.ascontiguousarray(
        np.asarray(from_depth_estimation, dtype=f32).transpose(1, 0, 2, 3)
        .reshape(C, N).astype(np.float16))
    w_dec32 = np.asarray(w_dec, dtype=f32)
    w_comb = w_dec32 @ np.asarray(w_rgb, dtype=f32)
    b_dec2 = w_dec32 @ np.asarray(b_rgb, dtype=f32) + np.asarray(b_dec, f32)
    base = {
        "d_full": d_cm, "x_tm": x_tm,
        "wT_lhs": np.ascontiguousarray(
            np.asarray(w_lhs, dtype=f32).T.astype(np.float16)),
        "w_rhs_om": np.ascontiguousarray(
            np.asarray(w_rhs, dtype=f32).astype(np.float16)),
        "wT_comb": np.ascontiguousarray(w_comb.T.astype(bf)),
        "b_lhs": np.asarray(b_lhs, dtype=f32),
        "b_dec2": b_dec2,
        "gamma": np.asarray(gamma, dtype=f32),
        "beta": np.asarray(beta, dtype=f32),
    }
    in_maps = []
    for i in range(NCORES):
        m = dict(base)
        m["d_q"] = np.ascontiguousarray(d_cm[:, i * Q:(i + 1) * Q])
        in_maps.append(m)
    return in_maps


def _assemble(results):
    out = np.empty((B, C, H, W), dtype=np.float32)
    rows = H // (NCORES // B)          # 24 rows of the image per core
    for i in range(NCORES):
        b, half = i // 2, i % 2
        out[b, :, half * rows:(half + 1) * rows, :] = (
            results[i]["y"].reshape(C, rows, W))
    return out


def kernel(x, from_depth_estimation, w_rgb, b_rgb, w_lhs, b_lhs, w_rhs, b_rhs,
           w_dec, b_dec, gamma, beta):
    global _nc_cache
    in_maps = _prepare_in_maps(x, from_depth_estimation, w_rgb, b_rgb, w_lhs,
                               b_lhs, w_rhs, b_rhs, w_dec, b_dec, gamma, beta)
    if _nc_cache is None:
        _nc_cache = _build()
    res = run_bass_kernel_spmd(_nc_cache, in_maps, list(range(NCORES)))
    return _assemble(res.results)
